# revision 26
# baseline (speedup 1.0000x reference)
"""Bass/Trainium2 kernel for nn_LocalAttention (banded attention, window 16).

Self-contained: takes full inputs, shards over 8 NeuronCores as
(batch, head-octet, seq-half), runs a banded-attention Bass kernel per core,
gathers on host.

Math: the reference zeroes out-of-band scores (not -inf) and softmaxes the
FULL row, so out-of-band entries contribute exp(0)=1.  With
em1 = band_mask(exp(s) - 1) (exactly 0 off-band and on padded keys):
  Z_i   = sum_window(em1) + S
  num_i = sum_window(em1 * v) + sum_all(v)

v4 design:
  * every matmul operand is bf16 (enables fast-weight-load, halves DMA)
  * all large inputs are host-pre-arranged to partition-major [128, ...]
    layout so each DMA is one contiguous descriptor per partition; loads
    are spread over five engine queues; output stores over three
  * dummy ones-matmuls warm the PE clock gate during the initial DMA wait
  * queries are tiled in blocks of 112 so each block's band covers exactly
    128 keys -> one scores matmul per block, no tail matmuls; the 10th
    block overlaps the 9th and only its non-overlapping tail is kept
  * 4 blocks share one [128,448] PSUM bank so the exp elementwise ops and
    the Z reduction run on 448-wide tiles
  * em1 = (exp(psum) - 1) * mask: exp straight off PSUM on the scalar
    engine, then one fused bf16 scalar_tensor_tensor on vector
  * Z = colsum(em1) via a single ones-lhsT matmul per bank + a rank-1
    matmul folding in the +S; even/odd heads land in partition halves
    0:64 / 64:128 of shared banks so the divide epilogue runs 128 wide
  * epilogue fused: ctx = (num + vsum) * recip(z) via scalar_tensor_tensor
  * scores are computed transposed ([keys, queries]) so em1 feeds the ctx
    matmul as rhs with V as the stationary operand
  * Q bias (and the 1/sqrt(HD) scale) folded into the psum->sbuf activation /
    host-side weights; K bias enters via an augmented ones-row of x that is
    zero on padded keys, so padding stays exact; bv/bo folded on the host
  * attention is software-pipelined (iteration i's scores issue before
    iteration i-1's z/ctx matmuls) and out-projection tiles are emitted
    into the attention stream as soon as their ctxt columns complete
"""
import os
import sys

for _p in ("/opt/trn_rl_repo",):
    if os.path.isdir(_p) and _p not in sys.path:
        sys.path.append(_p)

import numpy as np
import ml_dtypes

B, S, D = 2, 2048, 1024
H, HD = 16, 64
W = 16                    # band half-width 8
SC = 1024                 # seq chunk per core
HK = SC + W               # key halo chunk (1040)
HC = 512                  # head-dim columns per core (8 heads)
NH = HC // HD             # heads per core (8)
KD = D // 128             # contraction tiles (8)
QB = 112                  # query block (band spans exactly QB+16=128 keys)
QOFF = [112 * t for t in range(9)] + [SC - QB]          # 10 blocks, last overlaps
BANKS = [(0, 1, 2, 3), (4, 5, 6, 7), (8, 9)]            # psum bank packing

_CACHE = {}


def _build():
    import concourse.bacc as bacc
    import concourse.tile as tile
    from concourse import mybir

    f32 = mybir.dt.float32
    bf16 = mybir.dt.bfloat16
    Exp = mybir.ActivationFunctionType.Exp
    Copy = mybir.ActivationFunctionType.Copy
    Ident = mybir.ActivationFunctionType.Identity
    mult = mybir.AluOpType.mult
    subtract = mybir.AluOpType.subtract
    mm_add = mybir.AluOpType.add

    nc = bacc.Bacc("TRN2", target_bir_lowering=False, debug=False, num_devices=8)

    xt = nc.dram_tensor("xt", [128, KD * HK], bf16, kind="ExternalInput").ap()
    wq = nc.dram_tensor("wq", [128, KD * HC], bf16, kind="ExternalInput").ap()
    wk = nc.dram_tensor("wk", [128, KD * HC], bf16, kind="ExternalInput").ap()
    wv = nc.dram_tensor("wv", [128, KD * HC], bf16, kind="ExternalInput").ap()
    wo = nc.dram_tensor("wo", [128, (HC // 128) * D], bf16, kind="ExternalInput").ap()
    bqc = nc.dram_tensor("bqc", [128, 4], f32, kind="ExternalInput").ap()
    bkc = nc.dram_tensor("bkc", [128, 4], f32, kind="ExternalInput").ap()
    vsc = nc.dram_tensor("vsc", [128, 4], f32, kind="ExternalInput").ap()
    maskt = nc.dram_tensor("maskt", [128, 3, 448], bf16, kind="ExternalInput").ap()
    out = nc.dram_tensor("out", [SC, D], f32, kind="ExternalOutput").ap()

    KCH = [(0, 352), (352, 352), (704, 336)]   # kt psum column chunks

    with tile.TileContext(nc) as tc:
        with tc.tile_pool(name="stat", bufs=1) as stat, \
             tc.tile_pool(name="acts", bufs=1) as acts, \
             tc.tile_pool(name="wrk", bufs=3) as wrk, \
             tc.tile_pool(name="emp", bufs=4) as emp, \
             tc.tile_pool(name="zrp", bufs=2) as zrp, \
             tc.tile_pool(name="pmm", bufs=2, space="PSUM") as pmm, \
             tc.tile_pool(name="pst", bufs=3, space="PSUM") as pst, \
             tc.tile_pool(name="pzb", bufs=1, space="PSUM") as pzb, \
             tc.tile_pool(name="pcc", bufs=2, space="PSUM") as pcc:

            # ---- static inputs -> SBUF (one contiguous DMA per tensor,
            #      spread over three hwdge queues). Tile's cross-engine
            #      semaphores are cumulative per engine, so each queue's
            #      issues are strictly in consumption order and the memsets
            #      come before any gpsimd DMA.
            onesm = stat.tile([128, 64], bf16)
            nc.gpsimd.memset(onesm[:], 1.0)
            srow = stat.tile([1, 128], bf16)
            nc.gpsimd.memset(srow[:], float(S))
            orow = stat.tile([1, 448], bf16)
            nc.gpsimd.memset(orow[:], 1.0)
            ones512 = stat.tile([128, 512], bf16)
            nc.gpsimd.memset(ones512[:], 1.0)
            # flat 2D DMA views (one big contiguous descriptor per partition);
            # Q-critical xt is split over the two earliest-arming rings
            bqc_sb = stat.tile([128, 4], f32)
            nc.gpsimd.dma_start(bqc_sb[:], bqc)
            xt_sb = stat.tile([128, KD * HK], bf16)
            nc.gpsimd.dma_start(xt_sb[:, 0:4 * HK], xt[:, 0:4 * HK])
            wq_sb = stat.tile([128, KD * HC], bf16)
            nc.scalar.dma_start(wq_sb[:], wq)
            nc.scalar.dma_start(xt_sb[:, 4 * HK:8 * HK], xt[:, 4 * HK:8 * HK])
            wk_sb = stat.tile([128, KD * HC], bf16)
            nc.gpsimd.dma_start(wk_sb[:], wk)
            bkc_sb = stat.tile([128, 4], f32)
            nc.gpsimd.dma_start(bkc_sb[:], bkc)
            mask_sb = stat.tile([128, 3, 448], bf16)
            nc.gpsimd.dma_start(mask_sb[:], maskt)
            vsc_sb = stat.tile([128, 4], f32)
            nc.gpsimd.dma_start(vsc_sb[:], vsc)
            wv_sb = stat.tile([128, KD * HC], bf16)
            nc.sync.dma_start(wv_sb[:], wv)
            wo_sb = stat.tile([128, (HC // 128) * D], bf16)
            nc.sync.dma_start(wo_sb[:], wo)

            qt = acts.tile([128, 4, SC], bf16)     # Q^T (scale+bias folded)
            kt = acts.tile([128, 4, HK], bf16)     # K^T over halo keys
            vaug = acts.tile([128, 10, HC], bf16)  # V rows per query block
            ctxt = acts.tile([128, 4, SC], bf16)   # ctx^T

            # ---- PE clock-gate warmup bridging the input-DMA wait ----
            for _ in range(20):
                wps = pst.tile([128, 448], f32, tag="st")
                nc.tensor.matmul(wps[:64, :448], onesm[:, 0:64],
                                 ones512[:, 0:448], start=True, stop=True)

            # ---- Q^T = (Wq/8)^T x + bq/8 (bias via activation) ----
            for m in range(4):
                for n in range(2):
                    ps = pmm.tile([128, 512], f32, tag="mm")
                    for k in range(KD):
                        nc.tensor.matmul(
                            ps[:], wq_sb[:, k * HC + m * 128:k * HC + (m + 1) * 128],
                            xt_sb[:, k * HK + 8 + n * 512:k * HK + 8 + (n + 1) * 512],
                            start=(k == 0), stop=(k == KD - 1))
                    nc.scalar.activation(
                        qt[:, m, n * 512:(n + 1) * 512], ps[:], Ident,
                        bias=bqc_sb[:, m:m + 1])

            # ---- K^T over all HK halo keys; bias via activation (pad keys
            #      get a spurious bias, but the per-core mask zeroes their
            #      em1 rows, so padding stays exact) ----
            for m in range(4):
                for (c0, cw) in KCH:
                    ps = pmm.tile([128, 512], f32, tag="mm")
                    for k in range(KD):
                        nc.tensor.matmul(
                            ps[:, :cw], wk_sb[:, k * HC + m * 128:k * HC + (m + 1) * 128],
                            xt_sb[:, k * HK + c0:k * HK + c0 + cw],
                            start=(k == 0), stop=(k == KD - 1))
                    nc.scalar.activation(kt[:, m, c0:c0 + cw], ps[:, :cw],
                                         Ident, bias=bkc_sb[:, m:m + 1])

            # ---- V rows per query block (keys qo-8 .. qo+120) ----
            for t in range(10):
                qo = QOFF[t]
                ps = pmm.tile([128, 512], f32, tag="mm")
                for k in range(KD):
                    nc.tensor.matmul(
                        ps[:], xt_sb[:, k * HK + qo:k * HK + qo + 128],
                        wv_sb[:, k * HC:(k + 1) * HC],
                        start=(k == 0), stop=(k == KD - 1))
                nc.vector.tensor_copy(vaug[:, t, :], ps[:])

            # ---- banded attention + interleaved out projection ----
            out_engines = [nc.sync, nc.gpsimd]

            def emit_out(st_i):
                for nch in range(D // 512):
                    ps = pmm.tile([128, 512], f32, tag="mm")
                    for k4 in range(HC // 128):
                        nc.tensor.matmul(
                            ps[:], ctxt[:, k4, st_i * 128:(st_i + 1) * 128],
                            wo_sb[:, k4 * D + nch * 512:k4 * D + (nch + 1) * 512],
                            start=(k4 == 0), stop=(k4 == HC // 128 - 1))
                    o_sb = wrk.tile([128, 512], f32, tag="ob")
                    if nch == 0:
                        nc.scalar.activation(o_sb[:], ps[:], Copy)
                    else:
                        nc.vector.tensor_copy(o_sb[:], ps[:])
                    eng = out_engines[(st_i * 2 + nch) % 2]
                    eng.dma_start(
                        out[st_i * 128:(st_i + 1) * 128,
                            nch * 512:(nch + 1) * 512], o_sb[:])

            def emit_front(hp, g):
                """scores matmuls + exp/mask path for both heads."""
                blocks = BANKS[g]
                cw = len(blocks) * QB
                ems = []
                for hi in range(2):
                    hr = hi * 64
                    pT = pst.tile([128, 448], f32, tag="st")
                    for j, t in enumerate(blocks):
                        qo = QOFF[t]
                        nc.tensor.matmul(
                            pT[:, j * QB:(j + 1) * QB],
                            kt[hr:hr + 64, hp, qo:qo + 128],
                            qt[hr:hr + 64, hp, qo:qo + QB],
                            start=True, stop=True)
                    w1 = wrk.tile([128, 448], bf16, tag="w1")
                    em = emp.tile([128, 448], bf16, tag="em")
                    nc.scalar.activation(w1[:, :cw], pT[:, :cw], Exp)
                    nc.vector.scalar_tensor_tensor(
                        em[:, :cw], w1[:, :cw], 1.0, mask_sb[:, g, :cw],
                        subtract, mult)
                    ems.append(em)
                return (hp, g, ems)

            def emit_back(state):
                """z + ctx matmuls and divide epilogue for a completed front."""
                hp, g, ems = state
                blocks = BANKS[g]
                cw = len(blocks) * QB
                zb = pzb.tile([128, 448], f32, tag="zb")
                pc = pcc.tile([128, 448], f32, tag="cc")
                for hi in range(2):
                    h = 2 * hp + hi
                    hr = hi * 64
                    em = ems[hi]
                    nc.tensor.matmul(zb[hr:hr + 64, :cw], onesm[:, 0:64],
                                     em[:, :cw], start=True, stop=False)
                    for j, t in enumerate(blocks):
                        nc.tensor.matmul(
                            pc[hr:hr + 64, j * QB:(j + 1) * QB],
                            vaug[:, t, h * 64:(h + 1) * 64],
                            em[:, j * QB:(j + 1) * QB],
                            start=True, stop=True)
                # fold the +S into PSUM with a rank-1 matmul over both halves
                nc.tensor.matmul(zb[:, :cw], srow[0:1, :], orow[0:1, :cw],
                                 start=False, stop=True)
                rz = zrp.tile([128, 448], f32, tag="rz")
                nc.vector.reciprocal_approx_fast(rz[:, :cw], zb[:, :cw])
                vs = vsc_sb[:, hp:hp + 1]
                if g < 2:
                    nc.vector.scalar_tensor_tensor(
                        ctxt[:, hp, g * 448:(g + 1) * 448], pc[:, :448],
                        vs, rz[:, :448], mm_add, mult)
                else:
                    nc.vector.scalar_tensor_tensor(
                        ctxt[:, hp, 896:1008], pc[:, 0:QB], vs,
                        rz[:, 0:QB], mm_add, mult)
                    nc.vector.scalar_tensor_tensor(
                        ctxt[:, hp, 1008:1024], pc[:, 208:224], vs,
                        rz[:, 208:224], mm_add, mult)

            # g-outer order so early query columns of ctxt complete first;
            # after back #4+i (i>=1) out-tile i-1 is ready: g=0 finishes all
            # hp at back 4 (queries 0:448), g=1 at back 8 (448:896), g=2 at
            # back 12 (896:1024); emitting one st-tile (128 queries) per back
            # from back 5 on keeps every out matmul behind its ctxt writes.
            iters = [(hp, g) for g in range(len(BANKS)) for hp in range(4)]
            pending = None
            backs = 0
            next_st = 0
            for it in iters:
                st = emit_front(*it)
                if pending is not None:
                    emit_back(pending)
                    backs += 1
                    if backs >= 5 and next_st < backs - 4:
                        emit_out(next_st)
                        next_st += 1
                pending = st
            emit_back(pending)
            while next_st < SC // 128:
                emit_out(next_st)
                next_st += 1

    nc.compile()
    return nc


def _get_nc():
    if "nc" not in _CACHE:
        _CACHE["nc"] = _build()
    return _CACHE["nc"]


LAST_EXEC_NS = None


def _band_maskt(sh):
    """[128, 3, 448] bf16 per-bank transposed-window masks.

    Key row k holds key (qo - 8 + k); query col j is query (qo + j);
    in-band iff |j + 8 - k| <= 8  iff  j <= k <= j + 16.  Padded keys
    (left pad for the first seq-half's block 0, right pad for the second
    half's block 9) get their rows zeroed so the activation-applied K bias
    cannot leak into em1 there.
    """
    k = np.arange(128)[:, None]
    j = np.arange(QB)[None, :]
    m1 = ((k >= j) & (k <= j + W)).astype(np.float32)
    m = np.tile(m1, (1, 12)).reshape(128, 3, 448).copy()
    if sh == 0:
        m[0:8, 0, 0:QB] = 0.0          # block 0 keys -8..0 are pads
    else:
        m[120:128, 2, QB:2 * QB] = 0.0  # block 9 keys 1024..1032 are pads
    return m.astype(ml_dtypes.bfloat16)


def _pmaj(a, ko):
    """[ko*128, F] -> partition-major [128, ko, F] contiguous."""
    return np.ascontiguousarray(a.reshape(ko, 128, -1).transpose(1, 0, 2))


def kernel(hidden_states, Wq, bq, Wk, bk, Wv, bv, Wo, bo):
    global LAST_EXEC_NS
    from concourse.bass_utils import run_bass_kernel_spmd

    bf = ml_dtypes.bfloat16
    hs = np.asarray(hidden_states, dtype=np.float32)
    Wq, Wk, Wv, Wo = (np.asarray(a, dtype=np.float32) for a in (Wq, Wk, Wv, Wo))
    bq, bk, bv, bo = (np.asarray(a, dtype=np.float32) for a in (bq, bk, bv, bo))

    xpad = np.zeros((B, S + W, D), np.float32)
    xpad[:, 8:8 + S] = hs
    xT = np.ascontiguousarray(xpad.transpose(0, 2, 1)).astype(bf)  # [B,D,S+W]

    Wq8 = (Wq * 0.125).astype(bf)
    bq8 = bq * 0.125
    masks = [_band_maskt(0), _band_maskt(1)]

    in_maps = []
    for core in range(8):
        b, hg, sh = core // 4, (core // 2) % 2, core % 2
        cols = slice(hg * HC, (hg + 1) * HC)
        vs = xpad[b].sum(0, dtype=np.float64) @ Wv[:, cols].astype(np.float64)
        vs = vs.astype(np.float32)
        vsc = np.empty((128, 4), np.float32)
        bqcm = np.empty((128, 4), np.float32)
        bkcm = np.empty((128, 4), np.float32)
        for hp in range(4):
            vsc[0:64, hp] = vs[(2 * hp) * 64:(2 * hp + 1) * 64]
            vsc[64:128, hp] = vs[(2 * hp + 1) * 64:(2 * hp + 2) * 64]
        bqs = bq8[hg * HC:(hg + 1) * HC]
        bks = bk[hg * HC:(hg + 1) * HC]
        for m in range(4):
            bqcm[:, m] = bqs[m * 128:(m + 1) * 128]
            bkcm[:, m] = bks[m * 128:(m + 1) * 128]
        xc = np.ascontiguousarray(xT[b][:, sh * SC: sh * SC + HK])
        in_maps.append({
            "xt": _pmaj(xc, KD),
            "wq": _pmaj(np.ascontiguousarray(Wq8[:, cols]), KD),
            "wk": _pmaj(np.ascontiguousarray(Wk[:, cols].astype(bf)), KD),
            "wv": _pmaj(np.ascontiguousarray(Wv[:, cols].astype(bf)), KD),
            "wo": _pmaj(np.ascontiguousarray(Wo[cols, :].astype(bf)), HC // 128),
            "bqc": bqcm,
            "bkc": bkcm,
            "vsc": vsc,
            "maskt": masks[sh],
        })

    nc = _get_nc()
    trace_dir = os.environ.get("KERNEL_TRACE_DIR")
    kwargs = {}
    if trace_dir:
        kwargs = dict(trace=True, trace_cores=[0], tmpdir=trace_dir)
    res = run_bass_kernel_spmd(nc, in_maps, list(range(8)), **kwargs)
    LAST_EXEC_NS = res.exec_time_ns

    const = (bv.astype(np.float64) @ Wo.astype(np.float64)
             + bo.astype(np.float64)).astype(np.float32)
    outp = np.empty((B, S, D), np.float32)
    for b in range(B):
        for sh in range(2):
            acc = (res.results[4 * b + sh]["out"]
                   + res.results[4 * b + 2 + sh]["out"] + const)
            outp[b, sh * SC:(sh + 1) * SC] = acc
    return outp


# revision 27
# speedup vs baseline: 1.0546x; 1.0546x over previous
"""Bass/Trainium2 kernel for nn_LocalAttention (banded attention, window 16).

Self-contained: takes full inputs, shards over 8 NeuronCores as
(batch, head-octet, seq-half), runs a banded-attention Bass kernel per core,
gathers on host.

Math: the reference zeroes out-of-band scores (not -inf) and softmaxes the
FULL row, so out-of-band entries contribute exp(0)=1.  With
em1 = band_mask(exp(s) - 1) (exactly 0 off-band and on padded keys):
  Z_i   = sum_window(em1) + S
  num_i = sum_window(em1 * v) + sum_all(v)

v4 design:
  * every matmul operand is bf16 (enables fast-weight-load, halves DMA)
  * all large inputs are host-pre-arranged to partition-major [128, ...]
    layout so each DMA is one contiguous descriptor per partition; loads
    are spread over five engine queues; output stores over three
  * dummy ones-matmuls warm the PE clock gate during the initial DMA wait
  * queries are tiled in blocks of 112 so each block's band covers exactly
    128 keys -> one scores matmul per block, no tail matmuls; the 10th
    block overlaps the 9th and only its non-overlapping tail is kept
  * 4 blocks share one [128,448] PSUM bank so the exp elementwise ops and
    the Z reduction run on 448-wide tiles
  * em1 = (exp(psum) - 1) * mask: exp straight off PSUM on the scalar
    engine, then one fused bf16 scalar_tensor_tensor on vector
  * Z = colsum(em1) via a single ones-lhsT matmul per bank + a rank-1
    matmul folding in the +S; even/odd heads land in partition halves
    0:64 / 64:128 of shared banks so the divide epilogue runs 128 wide
  * epilogue fused: ctx = (num + vsum) * recip(z) via scalar_tensor_tensor
  * scores are computed transposed ([keys, queries]) so em1 feeds the ctx
    matmul as rhs with V as the stationary operand
  * Q bias (and the 1/sqrt(HD) scale) folded into the psum->sbuf activation /
    host-side weights; K bias enters via an augmented ones-row of x that is
    zero on padded keys, so padding stays exact; bv/bo folded on the host
  * attention is software-pipelined (iteration i's scores issue before
    iteration i-1's z/ctx matmuls) and out-projection tiles are emitted
    into the attention stream as soon as their ctxt columns complete
"""
import os
import sys

for _p in ("/opt/trn_rl_repo",):
    if os.path.isdir(_p) and _p not in sys.path:
        sys.path.append(_p)

import numpy as np
import ml_dtypes

B, S, D = 2, 2048, 1024
H, HD = 16, 64
W = 16                    # band half-width 8
SC = 1024                 # seq chunk per core
HK = SC + W               # key halo chunk (1040)
HC = 512                  # head-dim columns per core (8 heads)
NH = HC // HD             # heads per core (8)
KD = D // 128             # contraction tiles (8)
QB = 112                  # query block (band spans exactly QB+16=128 keys)
QOFF = [112 * t for t in range(9)] + [SC - QB]          # 10 blocks, last overlaps
BANKS = [(0, 1, 2, 3), (4, 5, 6, 7), (8, 9)]            # psum bank packing

_CACHE = {}


def _build():
    import concourse.bacc as bacc
    import concourse.tile as tile
    from concourse import mybir

    f32 = mybir.dt.float32
    bf16 = mybir.dt.bfloat16
    Exp = mybir.ActivationFunctionType.Exp
    Copy = mybir.ActivationFunctionType.Copy
    Ident = mybir.ActivationFunctionType.Identity
    mult = mybir.AluOpType.mult
    subtract = mybir.AluOpType.subtract
    mm_add = mybir.AluOpType.add

    nc = bacc.Bacc("TRN2", target_bir_lowering=False, debug=False, num_devices=8)

    xt = nc.dram_tensor("xt", [128, KD * HK], bf16, kind="ExternalInput").ap()
    wq = nc.dram_tensor("wq", [128, KD * HC], bf16, kind="ExternalInput").ap()
    wk = nc.dram_tensor("wk", [128, KD * HC], bf16, kind="ExternalInput").ap()
    wv = nc.dram_tensor("wv", [128, KD * HC], bf16, kind="ExternalInput").ap()
    wo = nc.dram_tensor("wo", [128, (HC // 128) * D], bf16, kind="ExternalInput").ap()
    bqc = nc.dram_tensor("bqc", [128, 4], f32, kind="ExternalInput").ap()
    bkc = nc.dram_tensor("bkc", [128, 4], f32, kind="ExternalInput").ap()
    vsc = nc.dram_tensor("vsc", [128, 4], f32, kind="ExternalInput").ap()
    maskt = nc.dram_tensor("maskt", [128, 3, 448], bf16, kind="ExternalInput").ap()
    out = nc.dram_tensor("out", [SC, D], f32, kind="ExternalOutput").ap()

    KCH = [(0, 352), (352, 352), (704, 336)]   # kt psum column chunks

    with tile.TileContext(nc) as tc:
        with tc.tile_pool(name="stat", bufs=1) as stat, \
             tc.tile_pool(name="acts", bufs=1) as acts, \
             tc.tile_pool(name="wrk", bufs=3) as wrk, \
             tc.tile_pool(name="emp", bufs=4) as emp, \
             tc.tile_pool(name="zrp", bufs=2) as zrp, \
             tc.tile_pool(name="pmm", bufs=2, space="PSUM") as pmm, \
             tc.tile_pool(name="pst", bufs=3, space="PSUM") as pst, \
             tc.tile_pool(name="pzb", bufs=1, space="PSUM") as pzb, \
             tc.tile_pool(name="pcc", bufs=2, space="PSUM") as pcc:

            # ---- static inputs -> SBUF (one contiguous DMA per tensor,
            #      spread over three hwdge queues). Tile's cross-engine
            #      semaphores are cumulative per engine, so each queue's
            #      issues are strictly in consumption order and the memsets
            #      come before any gpsimd DMA.
            onesm = stat.tile([128, 64], bf16)
            nc.gpsimd.memset(onesm[:], 1.0)
            srow = stat.tile([1, 128], bf16)
            nc.gpsimd.memset(srow[:], float(S))
            orow = stat.tile([1, 448], bf16)
            nc.gpsimd.memset(orow[:], 1.0)
            ones512 = stat.tile([128, 512], bf16)
            nc.gpsimd.memset(ones512[:], 1.0)
            # flat 2D DMA views (one big contiguous descriptor per partition)
            xt_sb = stat.tile([128, KD * HK], bf16)
            nc.sync.dma_start(xt_sb[:, 0:4 * HK], xt[:, 0:4 * HK])
            wq_sb = stat.tile([128, KD * HC], bf16)
            nc.scalar.dma_start(wq_sb[:], wq)
            bqc_sb = stat.tile([128, 4], f32)
            nc.gpsimd.dma_start(bqc_sb[:], bqc)
            nc.gpsimd.dma_start(xt_sb[:, 4 * HK:8 * HK], xt[:, 4 * HK:8 * HK])
            wk_sb = stat.tile([128, KD * HC], bf16)
            nc.gpsimd.dma_start(wk_sb[:], wk)
            bkc_sb = stat.tile([128, 4], f32)
            nc.gpsimd.dma_start(bkc_sb[:], bkc)
            mask_sb = stat.tile([128, 3, 448], bf16)
            nc.gpsimd.dma_start(mask_sb[:], maskt)
            vsc_sb = stat.tile([128, 4], f32)
            nc.gpsimd.dma_start(vsc_sb[:], vsc)
            wv_sb = stat.tile([128, KD * HC], bf16)
            nc.gpsimd.dma_start(wv_sb[:], wv)
            wo_sb = stat.tile([128, (HC // 128) * D], bf16)
            nc.scalar.dma_start(wo_sb[:], wo)

            qt = acts.tile([128, 4, SC], bf16)     # Q^T (scale+bias folded)
            kt = acts.tile([128, 4, HK], bf16)     # K^T over halo keys
            vaug = acts.tile([128, 10, HC], bf16)  # V rows per query block
            ctxt = acts.tile([128, 4, SC], bf16)   # ctx^T

            # ---- PE clock-gate warmup bridging the input-DMA wait ----
            for _ in range(20):
                wps = pst.tile([128, 448], f32, tag="st")
                nc.tensor.matmul(wps[:64, :448], onesm[:, 0:64],
                                 ones512[:, 0:448], start=True, stop=True)

            # ---- Q^T = (Wq/8)^T x + bq/8 (bias via activation) ----
            for m in range(4):
                for n in range(2):
                    ps = pmm.tile([128, 512], f32, tag="mm")
                    for k in range(KD):
                        nc.tensor.matmul(
                            ps[:], wq_sb[:, k * HC + m * 128:k * HC + (m + 1) * 128],
                            xt_sb[:, k * HK + 8 + n * 512:k * HK + 8 + (n + 1) * 512],
                            start=(k == 0), stop=(k == KD - 1))
                    nc.scalar.activation(
                        qt[:, m, n * 512:(n + 1) * 512], ps[:], Ident,
                        bias=bqc_sb[:, m:m + 1])

            # ---- K^T over all HK halo keys; bias via activation (pad keys
            #      get a spurious bias, but the per-core mask zeroes their
            #      em1 rows, so padding stays exact) ----
            for m in range(4):
                for (c0, cw) in KCH:
                    ps = pmm.tile([128, 512], f32, tag="mm")
                    for k in range(KD):
                        nc.tensor.matmul(
                            ps[:, :cw], wk_sb[:, k * HC + m * 128:k * HC + (m + 1) * 128],
                            xt_sb[:, k * HK + c0:k * HK + c0 + cw],
                            start=(k == 0), stop=(k == KD - 1))
                    nc.scalar.activation(kt[:, m, c0:c0 + cw], ps[:, :cw],
                                         Ident, bias=bkc_sb[:, m:m + 1])

            # ---- V rows per query block (keys qo-8 .. qo+120) ----
            for t in range(10):
                qo = QOFF[t]
                ps = pmm.tile([128, 512], f32, tag="mm")
                for k in range(KD):
                    nc.tensor.matmul(
                        ps[:], xt_sb[:, k * HK + qo:k * HK + qo + 128],
                        wv_sb[:, k * HC:(k + 1) * HC],
                        start=(k == 0), stop=(k == KD - 1))
                nc.vector.tensor_copy(vaug[:, t, :], ps[:])

            # ---- banded attention + interleaved out projection ----
            out_engines = [nc.sync, nc.gpsimd]

            def emit_out(st_i):
                for nch in range(D // 512):
                    ps = pmm.tile([128, 512], f32, tag="mm")
                    for k4 in range(HC // 128):
                        nc.tensor.matmul(
                            ps[:], ctxt[:, k4, st_i * 128:(st_i + 1) * 128],
                            wo_sb[:, k4 * D + nch * 512:k4 * D + (nch + 1) * 512],
                            start=(k4 == 0), stop=(k4 == HC // 128 - 1))
                    o_sb = wrk.tile([128, 512], f32, tag="ob")
                    if nch == 0:
                        nc.scalar.activation(o_sb[:], ps[:], Copy)
                    else:
                        nc.vector.tensor_copy(o_sb[:], ps[:])
                    eng = out_engines[(st_i * 2 + nch) % 2]
                    eng.dma_start(
                        out[st_i * 128:(st_i + 1) * 128,
                            nch * 512:(nch + 1) * 512], o_sb[:])

            def emit_front(hp, g):
                """scores matmuls + exp/mask path for both heads."""
                blocks = BANKS[g]
                cw = len(blocks) * QB
                ems = []
                for hi in range(2):
                    hr = hi * 64
                    pT = pst.tile([128, 448], f32, tag="st")
                    for j, t in enumerate(blocks):
                        qo = QOFF[t]
                        nc.tensor.matmul(
                            pT[:, j * QB:(j + 1) * QB],
                            kt[hr:hr + 64, hp, qo:qo + 128],
                            qt[hr:hr + 64, hp, qo:qo + QB],
                            start=True, stop=True)
                    w1 = wrk.tile([128, 448], bf16, tag="w1")
                    em = emp.tile([128, 448], bf16, tag="em")
                    nc.scalar.activation(w1[:, :cw], pT[:, :cw], Exp)
                    nc.vector.scalar_tensor_tensor(
                        em[:, :cw], w1[:, :cw], 1.0, mask_sb[:, g, :cw],
                        subtract, mult)
                    ems.append(em)
                return (hp, g, ems)

            def emit_back(state):
                """z + ctx matmuls and divide epilogue for a completed front."""
                hp, g, ems = state
                blocks = BANKS[g]
                cw = len(blocks) * QB
                zb = pzb.tile([128, 448], f32, tag="zb")
                pc = pcc.tile([128, 448], f32, tag="cc")
                for hi in range(2):
                    h = 2 * hp + hi
                    hr = hi * 64
                    em = ems[hi]
                    nc.tensor.matmul(zb[hr:hr + 64, :cw], onesm[:, 0:64],
                                     em[:, :cw], start=True, stop=False)
                    for j, t in enumerate(blocks):
                        nc.tensor.matmul(
                            pc[hr:hr + 64, j * QB:(j + 1) * QB],
                            vaug[:, t, h * 64:(h + 1) * 64],
                            em[:, j * QB:(j + 1) * QB],
                            start=True, stop=True)
                # fold the +S into PSUM with a rank-1 matmul over both halves
                nc.tensor.matmul(zb[:, :cw], srow[0:1, :], orow[0:1, :cw],
                                 start=False, stop=True)
                rz = zrp.tile([128, 448], f32, tag="rz")
                nc.vector.reciprocal_approx_fast(rz[:, :cw], zb[:, :cw])
                vs = vsc_sb[:, hp:hp + 1]
                if g < 2:
                    nc.vector.scalar_tensor_tensor(
                        ctxt[:, hp, g * 448:(g + 1) * 448], pc[:, :448],
                        vs, rz[:, :448], mm_add, mult)
                else:
                    nc.vector.scalar_tensor_tensor(
                        ctxt[:, hp, 896:1008], pc[:, 0:QB], vs,
                        rz[:, 0:QB], mm_add, mult)
                    nc.vector.scalar_tensor_tensor(
                        ctxt[:, hp, 1008:1024], pc[:, 208:224], vs,
                        rz[:, 208:224], mm_add, mult)

            # g-outer order so early query columns of ctxt complete first;
            # after back #4+i (i>=1) out-tile i-1 is ready: g=0 finishes all
            # hp at back 4 (queries 0:448), g=1 at back 8 (448:896), g=2 at
            # back 12 (896:1024); emitting one st-tile (128 queries) per back
            # from back 5 on keeps every out matmul behind its ctxt writes.
            iters = [(hp, g) for g in range(len(BANKS)) for hp in range(4)]
            pending = None
            backs = 0
            next_st = 0
            for it in iters:
                st = emit_front(*it)
                if pending is not None:
                    emit_back(pending)
                    backs += 1
                    if backs >= 5 and next_st < backs - 4:
                        emit_out(next_st)
                        next_st += 1
                pending = st
            emit_back(pending)
            while next_st < SC // 128:
                emit_out(next_st)
                next_st += 1

    nc.compile()
    return nc


def _get_nc():
    if "nc" not in _CACHE:
        _CACHE["nc"] = _build()
    return _CACHE["nc"]


LAST_EXEC_NS = None


def _band_maskt(sh):
    """[128, 3, 448] bf16 per-bank transposed-window masks.

    Key row k holds key (qo - 8 + k); query col j is query (qo + j);
    in-band iff |j + 8 - k| <= 8  iff  j <= k <= j + 16.  Padded keys
    (left pad for the first seq-half's block 0, right pad for the second
    half's block 9) get their rows zeroed so the activation-applied K bias
    cannot leak into em1 there.
    """
    k = np.arange(128)[:, None]
    j = np.arange(QB)[None, :]
    m1 = ((k >= j) & (k <= j + W)).astype(np.float32)
    m = np.tile(m1, (1, 12)).reshape(128, 3, 448).copy()
    if sh == 0:
        m[0:8, 0, 0:QB] = 0.0          # block 0 keys -8..0 are pads
    else:
        m[120:128, 2, QB:2 * QB] = 0.0  # block 9 keys 1024..1032 are pads
    return m.astype(ml_dtypes.bfloat16)


def _pmaj(a, ko):
    """[ko*128, F] -> partition-major [128, ko, F] contiguous."""
    return np.ascontiguousarray(a.reshape(ko, 128, -1).transpose(1, 0, 2))


def kernel(hidden_states, Wq, bq, Wk, bk, Wv, bv, Wo, bo):
    global LAST_EXEC_NS
    from concourse.bass_utils import run_bass_kernel_spmd

    bf = ml_dtypes.bfloat16
    hs = np.asarray(hidden_states, dtype=np.float32)
    Wq, Wk, Wv, Wo = (np.asarray(a, dtype=np.float32) for a in (Wq, Wk, Wv, Wo))
    bq, bk, bv, bo = (np.asarray(a, dtype=np.float32) for a in (bq, bk, bv, bo))

    xpad = np.zeros((B, S + W, D), np.float32)
    xpad[:, 8:8 + S] = hs
    xT = np.ascontiguousarray(xpad.transpose(0, 2, 1)).astype(bf)  # [B,D,S+W]

    Wq8 = (Wq * 0.125).astype(bf)
    bq8 = bq * 0.125
    masks = [_band_maskt(0), _band_maskt(1)]

    in_maps = []
    for core in range(8):
        b, hg, sh = core // 4, (core // 2) % 2, core % 2
        cols = slice(hg * HC, (hg + 1) * HC)
        vs = xpad[b].sum(0, dtype=np.float64) @ Wv[:, cols].astype(np.float64)
        vs = vs.astype(np.float32)
        vsc = np.empty((128, 4), np.float32)
        bqcm = np.empty((128, 4), np.float32)
        bkcm = np.empty((128, 4), np.float32)
        for hp in range(4):
            vsc[0:64, hp] = vs[(2 * hp) * 64:(2 * hp + 1) * 64]
            vsc[64:128, hp] = vs[(2 * hp + 1) * 64:(2 * hp + 2) * 64]
        bqs = bq8[hg * HC:(hg + 1) * HC]
        bks = bk[hg * HC:(hg + 1) * HC]
        for m in range(4):
            bqcm[:, m] = bqs[m * 128:(m + 1) * 128]
            bkcm[:, m] = bks[m * 128:(m + 1) * 128]
        xc = np.ascontiguousarray(xT[b][:, sh * SC: sh * SC + HK])
        in_maps.append({
            "xt": _pmaj(xc, KD),
            "wq": _pmaj(np.ascontiguousarray(Wq8[:, cols]), KD),
            "wk": _pmaj(np.ascontiguousarray(Wk[:, cols].astype(bf)), KD),
            "wv": _pmaj(np.ascontiguousarray(Wv[:, cols].astype(bf)), KD),
            "wo": _pmaj(np.ascontiguousarray(Wo[cols, :].astype(bf)), HC // 128),
            "bqc": bqcm,
            "bkc": bkcm,
            "vsc": vsc,
            "maskt": masks[sh],
        })

    nc = _get_nc()
    trace_dir = os.environ.get("KERNEL_TRACE_DIR")
    kwargs = {}
    if trace_dir:
        kwargs = dict(trace=True, trace_cores=[0], tmpdir=trace_dir)
    res = run_bass_kernel_spmd(nc, in_maps, list(range(8)), **kwargs)
    LAST_EXEC_NS = res.exec_time_ns

    const = (bv.astype(np.float64) @ Wo.astype(np.float64)
             + bo.astype(np.float64)).astype(np.float32)
    outp = np.empty((B, S, D), np.float32)
    for b in range(B):
        for sh in range(2):
            acc = (res.results[4 * b + sh]["out"]
                   + res.results[4 * b + 2 + sh]["out"] + const)
            outp[b, sh * SC:(sh + 1) * SC] = acc
    return outp
